# revision 27
# baseline (speedup 1.0000x reference)
"""Trainium2 Bass kernel for nn_DecoderCell (GRU-style decoder cell).

Reference computation (per batch row):
    r      = sigmoid(x @ Wr.T + hprev @ Ur.T + c @ Cr.T + br)
    z      = sigmoid(x @ Wz.T + hprev @ Uz.T + c @ Cz.T + bz)
    h_prop = tanh  (x @ Wh.T + (r * hprev) @ Uh.T + c @ Ch.T + bh)
    out    = z * h_prop + (1 - z) * hprev

Shapes: B=8192, IN=1024, H=1024, c is [B, 2H].

Strategy:
  - Data-parallel across 8 NeuronCores: batch shard of 1024 rows per core,
    weights replicated. No collectives.
  - All compute in the "transposed domain": per core we hold
    AT = [x | hprev | c].T  -> [4096, 1024]  (k-major: contraction dim on
    SBUF partitions) and per-gate weights M_g = [W_g | U_g | C_g].T
    -> [4096, 1024].  Gate pre-activations come out as [H, B_s] tiles, so
    the r*hprev product needed by the Uh matmul is produced directly in
    k-major layout and no on-device transposes are required.
  - "fp8dr" mode (default): operands are split on the host into
    v ~= v_hi + v_lo, both float8_e4m3 (~7 effective mantissa bits
    combined), and the PE runs in DoubleRow mode (2 contraction planes per
    partition, 2x MAC rate). Each preactivation accumulates three product
    streams W_hi*A_hi + W_hi*A_lo + W_lo*A_hi in fp32 PSUM (the lo*lo term
    is ~1e-7 relative and dropped). 1.5x the matmul count at 2x rate
    beats fp16 by ~25%; weight loads amortize 4:1/2:1 over matmuls.
  - "float16" mode: straight fp16 operands, 1 col/cycle (fallback).
  - PE loop: per gate, 4 column-quarter groups of 4 PSUM banks each
    (2 j-tiles x 2 batch-slices of 512); 8-bank PSUM pool double-buffers
    groups; weight slabs stream from HBM with a prefetch pool.
  - Biases are fused into the sigmoid/tanh activation instruction.
  - A post-schedule BIR pass removes back-to-back identical LDWEIGHTS
    (the walrus invocation runs with ldw-opt disabled, so Tile emits one
    per matmul even when the stationary operand is unchanged).
"""

import sys

sys.path.insert(0, "/opt/trn_rl_repo")

import numpy as np
from contextlib import ExitStack

B = 8192
IN = 1024
H = 1024
NCORES = 8
BS = B // NCORES          # batch rows per core
KT = 32                   # 128-row k-tiles in the 4096-deep contraction
KP = KT // 2              # DoubleRow k-pair tiles (256 contraction rows each)
NB = BS // 512            # 512-wide moving slices per core
KSLAB = 4                 # k-tiles (or k-pairs) per weight-slab DMA

MM_MODE = "fp8dr"         # "fp8dr" or "float16"

_CACHE = {}


def _dedup_ldweights(nc, mybir):
    """Drop redundant InstLdweights from the scheduled BIR.

    Tile splits every non-fp32 matmul into an explicit LDWEIGHTS + MATMUL
    pair, and the walrus invocation runs with --enable-ldw-opt=false, so
    back-to-back matmuls that reuse the same stationary tile each pay a
    full weight reload. The PE array keeps its weights across matmuls, so
    an LDWEIGHTS identical to the immediately preceding one (and carrying
    no semaphore waits or updates) is a no-op — remove it.
    """
    removed = 0
    for f in nc.m.functions:
        for bb in f.blocks:
            keep = []
            prev_sig = None
            for inst in bb.instructions:
                tn = type(inst).__name__
                if getattr(inst, "engine", None) == mybir.EngineType.PE:
                    if tn == "InstLdweights":
                        sig = str(inst.ins[0]) if inst.ins else None
                        si = inst.sync_info
                        clean = si is None or (
                            len(si.on_wait) == 0 and len(si.on_update) == 0
                        )
                        if sig is not None and sig == prev_sig and clean:
                            removed += 1
                            continue
                        prev_sig = sig
                    elif tn in ("InstMatmult", "InstEventSemaphore", "InstNoOp"):
                        pass  # these don't disturb the loaded weights
                    else:
                        prev_sig = None
                keep.append(inst)
            bb.instructions[:] = keep
    return removed


def _build_nc_fp8dr():
    import concourse.bacc as bacc
    import concourse.tile as tile
    from concourse import mybir

    f32 = mybir.dt.float32
    f16 = mybir.dt.float16
    f8 = mybir.dt.float8e4
    DR = mybir.MatmulPerfMode.DoubleRow
    SIG = mybir.ActivationFunctionType.Sigmoid
    TANH = mybir.ActivationFunctionType.Tanh

    nc = bacc.Bacc("TRN2", target_bir_lowering=False, debug=False)

    # activations, plane-interleaved for DoubleRow: [kp][p][plane][b]
    at_d = {
        h: nc.dram_tensor(f"at_{h}", [KP, 128, 2, BS], f8, kind="ExternalInput")
        for h in ("hi", "lo")
    }
    # weights pre-packed on host: [jq, ks, p, dkp, pl, jcol] so one slab
    # DMA reads a fully contiguous [128, KSLAB*2*256] block
    w_d = {
        (g, h): nc.dram_tensor(
            f"w{g}_{h}", [4, KP // KSLAB, 128, KSLAB, 2, 256], f8, kind="ExternalInput"
        )
        for g in "rzh"
        for h in ("hi", "lo")
    }
    ht_d = nc.dram_tensor("ht", [8, 128, BS], f16, kind="ExternalInput")
    b_d = {
        g: nc.dram_tensor(f"b{g}", [128, 8], f32, kind="ExternalInput")
        for g in "rzh"
    }
    out_d = nc.dram_tensor("out_t", [H, BS], f32, kind="ExternalOutput")

    with tile.TileContext(nc) as tc:
        with ExitStack() as ctx:
            pp = ctx.enter_context(tc.tile_pool(name="persist", bufs=1))
            wp = ctx.enter_context(tc.tile_pool(name="wslab", bufs=8))
            rp = ctx.enter_context(tc.tile_pool(name="rtmp", bufs=4))
            hpp = ctx.enter_context(tc.tile_pool(name="hprop", bufs=4))
            op = ctx.enter_context(tc.tile_pool(name="otile", bufs=4))
            psp = ctx.enter_context(tc.tile_pool(name="ps", bufs=8, space="PSUM"))

            at_t = {
                h: [pp.tile([128, 2, BS], f8, tag=f"at{h}{k}", name=f"at{h}{k}")
                    for k in range(KP)]
                for h in ("hi", "lo")
            }
            # r*hprev, plane-interleaved, k-pairs 4..7 of the h-gate
            rh_t = {
                h: [pp.tile([128, 2, BS], f8, tag=f"rh{h}{j}", name=f"rh{h}{j}")
                    for j in range(4)]
                for h in ("hi", "lo")
            }
            ht_t = [pp.tile([128, BS], f16, tag=f"ht{j}", name=f"ht{j}") for j in range(8)]
            z_t = [
                [pp.tile([128, 512], f16, tag=f"z{j}_{b}", name=f"z{j}_{b}") for b in range(NB)]
                for j in range(8)
            ]
            bias_t = {g: pp.tile([128, 8], f32, tag=f"bias{g}", name=f"bias{g}") for g in "rzh"}
            for g in "rzh":
                nc.sync.dma_start(bias_t[g][:], b_d[g].ap()[:, :])
            for j in range(8):
                nc.sync.dma_start(ht_t[j][:], ht_d.ap()[j, :, :])

            at_loaded = {("hi", k): False for k in range(KP)}
            at_loaded.update({("lo", k): False for k in range(KP)})

            def ensure_at(h, kp):
                if not at_loaded[(h, kp)]:
                    nc.sync.dma_start(at_t[h][kp][:], at_d[h].ap()[kp, :, :, :])
                    at_loaded[(h, kp)] = True

            def moving(g, kp, h, b):
                # h-gate contracts k-pairs 4..7 against r*hprev
                if g == "h" and 4 <= kp < 8:
                    return rh_t[h][kp - 4][:, :, b * 512:(b + 1) * 512]
                ensure_at(h, kp)
                return at_t[h][kp][:, :, b * 512:(b + 1) * 512]

            # stream order per stationary: W_hi x (A_hi, A_lo), W_lo x (A_hi)
            STREAMS = [("hi", ("hi", "lo")), ("lo", ("hi",))]

            def do_group(g, jq):
                ps = {}
                for jl in range(2):
                    for b in range(NB):
                        ps[(jl, b)] = psp.tile([128, 512], f32, tag="ps",
                                               name=f"ps_{g}_{jq}_{jl}_{b}")
                slabs = {}
                for ks in range(KP // KSLAB):
                    kp0 = KSLAB * ks
                    for wh in ("hi", "lo"):
                        slab = wp.tile([128, KSLAB, 2, 256], f8, tag=f"w{wh}",
                                       name=f"w{wh}_{g}_{jq}_{ks}")
                        src = w_d[(g, wh)].ap()[jq, ks, :, :, :, :]
                        nc.sync.dma_start(slab[:], src)
                        slabs[wh] = slab
                    for dkp in range(KSLAB):
                        kp = kp0 + dkp
                        for jl in range(2):
                            for wh, ahs in STREAMS:
                                lhsT = slabs[wh][:, dkp, :, jl * 128:(jl + 1) * 128]
                                for ah in ahs:
                                    for b in range(NB):
                                        nc.tensor.matmul(
                                            ps[(jl, b)][:],
                                            lhsT,
                                            moving(g, kp, ah, b),
                                            start=(kp == 0 and wh == "hi" and ah == "hi"),
                                            stop=(kp == KP - 1 and wh == "lo"),
                                            perf_mode=DR,
                                        )
                for jl in range(2):
                    jt = 2 * jq + jl
                    for b in range(NB):
                        pst = ps[(jl, b)]
                        bias_ap = bias_t[g][:, jt:jt + 1]
                        bsl = slice(b * 512, (b + 1) * 512)
                        if g == "r":
                            # r -> rh = r*h -> fp8 hi/lo planes
                            tmp = rp.tile([128, 512], f32, tag="rt", name=f"rt_{jt}_{b}")
                            nc.scalar.activation(tmp[:], pst[:], SIG, bias=bias_ap)
                            rhf = rp.tile([128, 512], f32, tag="rhf", name=f"rhf_{jt}_{b}")
                            nc.vector.tensor_mul(rhf[:], tmp[:], ht_t[jt][:, bsl])
                            kp_l, pl = divmod(jt, 2)
                            hi_sl = rh_t["hi"][kp_l][:, pl, bsl]
                            lo_sl = rh_t["lo"][kp_l][:, pl, bsl]
                            nc.vector.tensor_copy(hi_sl, rhf[:])
                            nc.vector.tensor_sub(lo_sl, rhf[:], hi_sl)
                        elif g == "z":
                            nc.scalar.activation(z_t[jt][b][:], pst[:], SIG, bias=bias_ap)
                        else:
                            hp = hpp.tile([128, 512], f32, tag="hp", name=f"hp_{jt}_{b}")
                            nc.scalar.activation(hp[:], pst[:], TANH, bias=bias_ap)
                            hT = ht_t[jt][:, bsl]
                            ot = op.tile([128, 512], f32, tag="ot", name=f"ot_{jt}_{b}")
                            # out = h + z*(hp - h)
                            nc.vector.tensor_sub(ot[:], hp[:], hT)
                            nc.vector.tensor_mul(hp[:], z_t[jt][b][:], ot[:])
                            nc.vector.tensor_add(ot[:], hp[:], hT)
                            nc.sync.dma_start(
                                out_d.ap()[jt * 128:(jt + 1) * 128, bsl], ot[:]
                            )

            for g in ("r", "z", "h"):
                for jq in range(4):
                    do_group(g, jq)

    _dedup_ldweights(nc, mybir)
    nc.finalize()
    return nc


def _build_nc_f16():
    import concourse.bacc as bacc
    import concourse.tile as tile
    from concourse import mybir

    f32 = mybir.dt.float32
    f16 = mybir.dt.float16
    SIG = mybir.ActivationFunctionType.Sigmoid
    TANH = mybir.ActivationFunctionType.Tanh

    nc = bacc.Bacc("TRN2", target_bir_lowering=False, debug=False)

    at_d = nc.dram_tensor("at", [4 * IN, BS], f16, kind="ExternalInput")
    w_d = {
        g: nc.dram_tensor(f"w{g}", [4 * IN, H], f16, kind="ExternalInput")
        for g in "rzh"
    }
    b_d = {
        g: nc.dram_tensor(f"b{g}", [128, 8], f32, kind="ExternalInput")
        for g in "rzh"
    }
    out_d = nc.dram_tensor("out_t", [H, BS], f32, kind="ExternalOutput")

    with tile.TileContext(nc) as tc:
        with ExitStack() as ctx:
            pp = ctx.enter_context(tc.tile_pool(name="persist", bufs=1))
            wp = ctx.enter_context(tc.tile_pool(name="wslab", bufs=6))
            rp = ctx.enter_context(tc.tile_pool(name="rtmp", bufs=4))
            hpp = ctx.enter_context(tc.tile_pool(name="hprop", bufs=4))
            op = ctx.enter_context(tc.tile_pool(name="otile", bufs=4))
            psp = ctx.enter_context(tc.tile_pool(name="ps", bufs=8, space="PSUM"))

            at_t = [pp.tile([128, BS], f16, tag=f"at{k}", name=f"at{k}") for k in range(KT)]
            rh_t = [pp.tile([128, BS], f16, tag=f"rh{j}", name=f"rh{j}") for j in range(8)]
            z_t = [
                [pp.tile([128, 512], f16, tag=f"z{j}_{b}", name=f"z{j}_{b}") for b in range(NB)]
                for j in range(8)
            ]
            bias_t = {g: pp.tile([128, 8], f32, tag=f"bias{g}", name=f"bias{g}") for g in "rzh"}
            for g in "rzh":
                nc.sync.dma_start(bias_t[g][:], b_d[g].ap()[:, :])

            at_dma = [None] * KT

            def ensure_at(k):
                if at_dma[k] is None:
                    at_dma[k] = nc.sync.dma_start(
                        at_t[k][:], at_d.ap()[k * 128:(k + 1) * 128, :]
                    )
                return at_dma[k]

            def moving(g, k, b):
                if g == "h" and 8 <= k < 16:
                    return rh_t[k - 8][:, b * 512:(b + 1) * 512]
                return at_t[k][:, b * 512:(b + 1) * 512]

            def do_group(g, jq):
                ps = {}
                for jl in range(2):
                    for b in range(NB):
                        ps[(jl, b)] = psp.tile([128, 512], f32, tag="ps",
                                               name=f"ps_{g}_{jq}_{jl}_{b}")
                for ks in range(KT // KSLAB):
                    k0 = KSLAB * ks
                    slab = wp.tile([128, KSLAB, 256], f16, tag="w", name=f"w_{g}_{jq}_{ks}")
                    src = w_d[g].ap()[k0 * 128:(k0 + KSLAB) * 128,
                                      jq * 256:(jq + 1) * 256]
                    nc.sync.dma_start(slab[:], src.rearrange("(a p) j -> p a j", p=128))
                    for dk in range(KSLAB):
                        k = k0 + dk
                        if not (g == "h" and 8 <= k < 16):
                            ensure_at(k)
                        for jl in range(2):
                            lhsT = slab[:, dk, jl * 128:(jl + 1) * 128]
                            for b in range(NB):
                                nc.tensor.matmul(
                                    ps[(jl, b)][:],
                                    lhsT,
                                    moving(g, k, b),
                                    start=(k == 0),
                                    stop=(k == KT - 1),
                                )
                for jl in range(2):
                    jt = 2 * jq + jl
                    for b in range(NB):
                        pst = ps[(jl, b)]
                        bias_ap = bias_t[g][:, jt:jt + 1]
                        bsl = slice(b * 512, (b + 1) * 512)
                        if g == "r":
                            tmp = rp.tile([128, 512], f32, tag="rt", name=f"rt_{jt}_{b}")
                            nc.scalar.activation(tmp[:], pst[:], SIG, bias=bias_ap)
                            nc.vector.tensor_mul(
                                rh_t[jt][:, bsl], tmp[:], at_t[8 + jt][:, bsl]
                            )
                        elif g == "z":
                            nc.scalar.activation(z_t[jt][b][:], pst[:], SIG, bias=bias_ap)
                        else:
                            hp = hpp.tile([128, 512], f32, tag="hp", name=f"hp_{jt}_{b}")
                            nc.scalar.activation(hp[:], pst[:], TANH, bias=bias_ap)
                            hT = at_t[8 + jt][:, bsl]
                            ot = op.tile([128, 512], f32, tag="ot", name=f"ot_{jt}_{b}")
                            nc.vector.tensor_sub(ot[:], hp[:], hT)
                            nc.vector.tensor_mul(hp[:], z_t[jt][b][:], ot[:])
                            nc.vector.tensor_add(ot[:], hp[:], hT)
                            nc.sync.dma_start(
                                out_d.ap()[jt * 128:(jt + 1) * 128, bsl], ot[:]
                            )

            for g in ("r", "z", "h"):
                for jq in range(4):
                    do_group(g, jq)

    _dedup_ldweights(nc, mybir)
    nc.finalize()
    return nc


def _get_nc():
    if "nc" not in _CACHE:
        _CACHE["nc"] = _build_nc_fp8dr() if MM_MODE == "fp8dr" else _build_nc_f16()
    return _CACHE["nc"]


def _fp8_split(arr32):
    import ml_dtypes

    hi = arr32.astype(ml_dtypes.float8_e4m3fn)
    lo = (arr32 - hi.astype(np.float32)).astype(ml_dtypes.float8_e4m3fn)
    return hi, lo


def _plane_interleave(arr, ncols):
    # [4096, N] -> [KP, 128, 2, N] with plane = k-tile parity within the pair
    return np.ascontiguousarray(
        arr.reshape(KP, 2, 128, ncols).transpose(0, 2, 1, 3)
    )


def _pack_weight_slabs(arr):
    # [4096, 1024] -> [jq, ks, p, dkp, pl, jcol] slab-contiguous layout
    a = _plane_interleave(arr, H)                       # [KP, 128, 2, H]
    a = a.reshape(KP // KSLAB, KSLAB, 128, 2, 4, 256)   # [ks, dkp, p, pl, jq, jcol]
    return np.ascontiguousarray(a.transpose(4, 0, 2, 1, 3, 5))


def _host_prep(inputs):
    x = np.asarray(inputs["x"], dtype=np.float32)
    hprev = np.asarray(inputs["hprev"], dtype=np.float32)
    c = np.asarray(inputs["c"], dtype=np.float32)
    A = np.concatenate([x, hprev, c], axis=1)          # [B, 4096]
    AT = np.ascontiguousarray(A.T)                     # [4096, B] f32
    wT = {}
    for g, (W, U, C) in {
        "r": (inputs["Wr"], inputs["Ur"], inputs["Cr"]),
        "z": (inputs["Wz"], inputs["Uz"], inputs["Cz"]),
        "h": (inputs["Wh"], inputs["Uh"], inputs["Ch"]),
    }.items():
        M = np.concatenate(
            [np.asarray(W, np.float32), np.asarray(U, np.float32), np.asarray(C, np.float32)],
            axis=1,
        )
        wT[g] = np.ascontiguousarray(M.T)              # [4096, 1024] f32
    bias = {
        g: np.ascontiguousarray(
            np.asarray(inputs["b" + g], dtype=np.float32).reshape(8, 128).T
        )
        for g in "rzh"
    }
    return AT, wT, bias, hprev


def _in_maps(inputs):
    AT, wT, bias, hprev = _host_prep(inputs)
    maps = []
    if MM_MODE == "fp8dr":
        w_split = {}
        for g in "rzh":
            hi, lo = _fp8_split(wT[g])
            w_split[g] = (_pack_weight_slabs(hi), _pack_weight_slabs(lo))
        hT_full = np.ascontiguousarray(hprev.T.astype(np.float16))  # [1024, B]
        at_hi_f, at_lo_f = _fp8_split(AT)
        for s in range(NCORES):
            sl = slice(s * BS, (s + 1) * BS)
            m = {
                "at_hi": _plane_interleave(np.ascontiguousarray(at_hi_f[:, sl]), BS),
                "at_lo": _plane_interleave(np.ascontiguousarray(at_lo_f[:, sl]), BS),
                "ht": np.ascontiguousarray(hT_full[:, sl]).reshape(8, 128, BS),
                "br": bias["r"],
                "bz": bias["z"],
                "bh": bias["h"],
            }
            for g in "rzh":
                m[f"w{g}_hi"], m[f"w{g}_lo"] = w_split[g]
            maps.append(m)
    else:
        w16 = {g: wT[g].astype(np.float16) for g in "rzh"}
        AT16 = AT.astype(np.float16)
        for s in range(NCORES):
            maps.append(
                {
                    "at": np.ascontiguousarray(AT16[:, s * BS:(s + 1) * BS]),
                    "wr": w16["r"],
                    "wz": w16["z"],
                    "wh": w16["h"],
                    "br": bias["r"],
                    "bz": bias["z"],
                    "bh": bias["h"],
                }
            )
    return maps


def run_device(inputs, trace=False, **kwargs):
    """Run the SPMD kernel; returns (full_output, BassKernelResults)."""
    from concourse.bass_utils import run_bass_kernel_spmd

    nc = _get_nc()
    res = run_bass_kernel_spmd(
        nc, _in_maps(inputs), core_ids=list(range(NCORES)), trace=trace, **kwargs
    )
    out = np.empty((B, H), dtype=np.float32)
    for s in range(NCORES):
        out[s * BS:(s + 1) * BS, :] = res.results[s]["out_t"].T
    return out, res


def kernel(**inputs):
    out, _ = run_device(inputs, trace=False)
    return out


# revision 28
# speedup vs baseline: 1.4659x; 1.4659x over previous
"""Trainium2 Bass kernel for nn_DecoderCell (GRU-style decoder cell).

Reference computation (per batch row):
    r      = sigmoid(x @ Wr.T + hprev @ Ur.T + c @ Cr.T + br)
    z      = sigmoid(x @ Wz.T + hprev @ Uz.T + c @ Cz.T + bz)
    h_prop = tanh  (x @ Wh.T + (r * hprev) @ Uh.T + c @ Ch.T + bh)
    out    = z * h_prop + (1 - z) * hprev

Shapes: B=8192, IN=1024, H=1024, c is [B, 2H].

Strategy:
  - Data-parallel across 8 NeuronCores: batch shard of 1024 rows per core,
    weights replicated. No collectives.
  - All compute in the "transposed domain": per core we hold
    AT = [x | hprev | c].T  -> [4096, 1024]  (k-major: contraction dim on
    SBUF partitions) and per-gate weights M_g = [W_g | U_g | C_g].T
    -> [4096, 1024].  Gate pre-activations come out as [H, B_s] tiles, so
    the r*hprev product needed by the Uh matmul is produced directly in
    k-major layout and no on-device transposes are required.
  - "fp8dr" mode (default): operands are split on the host into
    v ~= v_hi + v_lo, both float8_e4m3 (~7 effective mantissa bits
    combined), and the PE runs in DoubleRow mode (2 contraction planes per
    partition, 2x MAC rate). Each preactivation accumulates three product
    streams W_hi*A_hi + W_hi*A_lo + W_lo*A_hi in fp32 PSUM (the lo*lo term
    is ~1e-7 relative and dropped). 1.5x the matmul count at 2x rate
    beats fp16 by ~25%; weight loads amortize 4:1/2:1 over matmuls.
  - "float16" mode: straight fp16 operands, 1 col/cycle (fallback).
  - PE loop: per gate, 4 column-quarter groups of 4 PSUM banks each
    (2 j-tiles x 2 batch-slices of 512); 8-bank PSUM pool double-buffers
    groups; weight slabs stream from HBM with a prefetch pool.
  - Biases are fused into the sigmoid/tanh activation instruction.
  - A post-schedule BIR pass removes back-to-back identical LDWEIGHTS
    (the walrus invocation runs with ldw-opt disabled, so Tile emits one
    per matmul even when the stationary operand is unchanged).
"""

import sys

sys.path.insert(0, "/opt/trn_rl_repo")

import numpy as np
from contextlib import ExitStack

B = 8192
IN = 1024
H = 1024
NCORES = 8
BS = B // NCORES          # batch rows per core
KT = 32                   # 128-row k-tiles in the 4096-deep contraction
KP = KT // 2              # DoubleRow k-pair tiles (256 contraction rows each)
NB = BS // 512            # 512-wide moving slices per core
KSLAB = 4                 # k-tiles (or k-pairs) per weight-slab DMA

import os
MM_MODE = os.environ.get("DECODER_MM_MODE", "float16")  # "fp8dr" or "float16"

_CACHE = {}


def _dedup_ldweights(nc, mybir):
    """Drop redundant InstLdweights from the scheduled BIR.

    Tile splits every non-fp32 matmul into an explicit LDWEIGHTS + MATMUL
    pair, and the walrus invocation runs with --enable-ldw-opt=false, so
    back-to-back matmuls that reuse the same stationary tile each pay a
    full weight reload. The PE array keeps its weights across matmuls, so
    an LDWEIGHTS identical to the immediately preceding one (and carrying
    no semaphore waits or updates) is a no-op — remove it.
    """
    removed = 0
    for f in nc.m.functions:
        for bb in f.blocks:
            keep = []
            prev_sig = None
            for inst in bb.instructions:
                tn = type(inst).__name__
                if getattr(inst, "engine", None) == mybir.EngineType.PE:
                    if tn == "InstLdweights":
                        sig = str(inst.ins[0]) if inst.ins else None
                        si = inst.sync_info
                        clean = si is None or (
                            len(si.on_wait) == 0 and len(si.on_update) == 0
                        )
                        if sig is not None and sig == prev_sig and clean:
                            removed += 1
                            continue
                        prev_sig = sig
                    elif tn in ("InstMatmult", "InstEventSemaphore", "InstNoOp"):
                        pass  # these don't disturb the loaded weights
                    else:
                        prev_sig = None
                keep.append(inst)
            bb.instructions[:] = keep
    return removed


def _build_nc_fp8dr():
    import concourse.bacc as bacc
    import concourse.tile as tile
    from concourse import mybir

    f32 = mybir.dt.float32
    f16 = mybir.dt.float16
    f8 = mybir.dt.float8e4
    DR = mybir.MatmulPerfMode.DoubleRow
    SIG = mybir.ActivationFunctionType.Sigmoid
    TANH = mybir.ActivationFunctionType.Tanh

    nc = bacc.Bacc("TRN2", target_bir_lowering=False, debug=False)

    # activations, plane-interleaved for DoubleRow: [kp][p][plane][b]
    at_d = {
        h: nc.dram_tensor(f"at_{h}", [KP, 128, 2, BS], f8, kind="ExternalInput")
        for h in ("hi", "lo")
    }
    # weights pre-packed on host: [jq, ks, p, dkp, pl, jcol] so one slab
    # DMA reads a fully contiguous [128, KSLAB*2*256] block
    w_d = {
        (g, h): nc.dram_tensor(
            f"w{g}_{h}", [4, KP // KSLAB, 128, KSLAB, 2, 256], f8, kind="ExternalInput"
        )
        for g in "rzh"
        for h in ("hi", "lo")
    }
    ht_d = nc.dram_tensor("ht", [8, 128, BS], f16, kind="ExternalInput")
    b_d = {
        g: nc.dram_tensor(f"b{g}", [128, 8], f32, kind="ExternalInput")
        for g in "rzh"
    }
    out_d = nc.dram_tensor("out_t", [H, BS], f32, kind="ExternalOutput")

    with tile.TileContext(nc) as tc:
        with ExitStack() as ctx:
            pp = ctx.enter_context(tc.tile_pool(name="persist", bufs=1))
            wp = ctx.enter_context(tc.tile_pool(name="wslab", bufs=8))
            rp = ctx.enter_context(tc.tile_pool(name="rtmp", bufs=4))
            hpp = ctx.enter_context(tc.tile_pool(name="hprop", bufs=4))
            op = ctx.enter_context(tc.tile_pool(name="otile", bufs=4))
            psp = ctx.enter_context(tc.tile_pool(name="ps", bufs=8, space="PSUM"))

            at_t = {
                h: [pp.tile([128, 2, BS], f8, tag=f"at{h}{k}", name=f"at{h}{k}")
                    for k in range(KP)]
                for h in ("hi", "lo")
            }
            # r*hprev, plane-interleaved, k-pairs 4..7 of the h-gate
            rh_t = {
                h: [pp.tile([128, 2, BS], f8, tag=f"rh{h}{j}", name=f"rh{h}{j}")
                    for j in range(4)]
                for h in ("hi", "lo")
            }
            ht_t = [pp.tile([128, BS], f16, tag=f"ht{j}", name=f"ht{j}") for j in range(8)]
            z_t = [
                [pp.tile([128, 512], f16, tag=f"z{j}_{b}", name=f"z{j}_{b}") for b in range(NB)]
                for j in range(8)
            ]
            bias_t = {g: pp.tile([128, 8], f32, tag=f"bias{g}", name=f"bias{g}") for g in "rzh"}
            for g in "rzh":
                nc.sync.dma_start(bias_t[g][:], b_d[g].ap()[:, :])
            for j in range(8):
                nc.sync.dma_start(ht_t[j][:], ht_d.ap()[j, :, :])

            at_loaded = {("hi", k): False for k in range(KP)}
            at_loaded.update({("lo", k): False for k in range(KP)})

            def ensure_at(h, kp):
                if not at_loaded[(h, kp)]:
                    nc.sync.dma_start(at_t[h][kp][:], at_d[h].ap()[kp, :, :, :])
                    at_loaded[(h, kp)] = True

            def moving(g, kp, h, b):
                # h-gate contracts k-pairs 4..7 against r*hprev
                if g == "h" and 4 <= kp < 8:
                    return rh_t[h][kp - 4][:, :, b * 512:(b + 1) * 512]
                ensure_at(h, kp)
                return at_t[h][kp][:, :, b * 512:(b + 1) * 512]

            # stream order per stationary: W_hi x (A_hi, A_lo), W_lo x (A_hi)
            STREAMS = [("hi", ("hi", "lo")), ("lo", ("hi",))]

            def do_group(g, jq):
                ps = {}
                for jl in range(2):
                    for b in range(NB):
                        ps[(jl, b)] = psp.tile([128, 512], f32, tag="ps",
                                               name=f"ps_{g}_{jq}_{jl}_{b}")
                slabs = {}
                for ks in range(KP // KSLAB):
                    kp0 = KSLAB * ks
                    for wh in ("hi", "lo"):
                        slab = wp.tile([128, KSLAB, 2, 256], f8, tag=f"w{wh}",
                                       name=f"w{wh}_{g}_{jq}_{ks}")
                        src = w_d[(g, wh)].ap()[jq, ks, :, :, :, :]
                        nc.sync.dma_start(slab[:], src)
                        slabs[wh] = slab
                    for dkp in range(KSLAB):
                        kp = kp0 + dkp
                        for jl in range(2):
                            for wh, ahs in STREAMS:
                                lhsT = slabs[wh][:, dkp, :, jl * 128:(jl + 1) * 128]
                                for ah in ahs:
                                    for b in range(NB):
                                        nc.tensor.matmul(
                                            ps[(jl, b)][:],
                                            lhsT,
                                            moving(g, kp, ah, b),
                                            start=(kp == 0 and wh == "hi" and ah == "hi"),
                                            stop=(kp == KP - 1 and wh == "lo"),
                                            perf_mode=DR,
                                        )
                for jl in range(2):
                    jt = 2 * jq + jl
                    for b in range(NB):
                        pst = ps[(jl, b)]
                        bias_ap = bias_t[g][:, jt:jt + 1]
                        bsl = slice(b * 512, (b + 1) * 512)
                        if g == "r":
                            # r -> rh = r*h -> fp8 hi/lo planes
                            tmp = rp.tile([128, 512], f32, tag="rt", name=f"rt_{jt}_{b}")
                            nc.scalar.activation(tmp[:], pst[:], SIG, bias=bias_ap)
                            rhf = rp.tile([128, 512], f32, tag="rhf", name=f"rhf_{jt}_{b}")
                            nc.vector.tensor_mul(rhf[:], tmp[:], ht_t[jt][:, bsl])
                            kp_l, pl = divmod(jt, 2)
                            hi_sl = rh_t["hi"][kp_l][:, pl, bsl]
                            lo_sl = rh_t["lo"][kp_l][:, pl, bsl]
                            nc.vector.tensor_copy(hi_sl, rhf[:])
                            nc.vector.tensor_sub(lo_sl, rhf[:], hi_sl)
                        elif g == "z":
                            nc.scalar.activation(z_t[jt][b][:], pst[:], SIG, bias=bias_ap)
                        else:
                            hp = hpp.tile([128, 512], f32, tag="hp", name=f"hp_{jt}_{b}")
                            nc.scalar.activation(hp[:], pst[:], TANH, bias=bias_ap)
                            hT = ht_t[jt][:, bsl]
                            ot = op.tile([128, 512], f32, tag="ot", name=f"ot_{jt}_{b}")
                            # out = h + z*(hp - h)
                            nc.vector.tensor_sub(ot[:], hp[:], hT)
                            nc.vector.tensor_mul(hp[:], z_t[jt][b][:], ot[:])
                            nc.vector.tensor_add(ot[:], hp[:], hT)
                            nc.sync.dma_start(
                                out_d.ap()[jt * 128:(jt + 1) * 128, bsl], ot[:]
                            )

            for g in ("r", "z", "h"):
                for jq in range(4):
                    do_group(g, jq)

    _dedup_ldweights(nc, mybir)
    nc.finalize()
    return nc


def _build_nc_f16():
    import concourse.bacc as bacc
    import concourse.tile as tile
    from concourse import mybir

    f32 = mybir.dt.float32
    f16 = mybir.dt.float16
    SIG = mybir.ActivationFunctionType.Sigmoid
    TANH = mybir.ActivationFunctionType.Tanh

    nc = bacc.Bacc("TRN2", target_bir_lowering=False, debug=False)

    at_d = nc.dram_tensor("at", [4 * IN, BS], f16, kind="ExternalInput")
    w_d = {
        g: nc.dram_tensor(f"w{g}", [4 * IN, H], f16, kind="ExternalInput")
        for g in "rzh"
    }
    b_d = {
        g: nc.dram_tensor(f"b{g}", [128, 8], f32, kind="ExternalInput")
        for g in "rzh"
    }
    out_d = nc.dram_tensor("out_t", [H, BS], f32, kind="ExternalOutput")

    with tile.TileContext(nc) as tc:
        with ExitStack() as ctx:
            pp = ctx.enter_context(tc.tile_pool(name="persist", bufs=1))
            wp = ctx.enter_context(tc.tile_pool(name="wslab", bufs=6))
            rp = ctx.enter_context(tc.tile_pool(name="rtmp", bufs=4))
            hpp = ctx.enter_context(tc.tile_pool(name="hprop", bufs=4))
            op = ctx.enter_context(tc.tile_pool(name="otile", bufs=4))
            psp = ctx.enter_context(tc.tile_pool(name="ps", bufs=8, space="PSUM"))

            at_t = [pp.tile([128, BS], f16, tag=f"at{k}", name=f"at{k}") for k in range(KT)]
            rh_t = [pp.tile([128, BS], f16, tag=f"rh{j}", name=f"rh{j}") for j in range(8)]
            z_t = [
                [pp.tile([128, 512], f16, tag=f"z{j}_{b}", name=f"z{j}_{b}") for b in range(NB)]
                for j in range(8)
            ]
            bias_t = {g: pp.tile([128, 8], f32, tag=f"bias{g}", name=f"bias{g}") for g in "rzh"}
            for g in "rzh":
                nc.sync.dma_start(bias_t[g][:], b_d[g].ap()[:, :])

            at_dma = [None] * KT

            def ensure_at(k):
                if at_dma[k] is None:
                    at_dma[k] = nc.sync.dma_start(
                        at_t[k][:], at_d.ap()[k * 128:(k + 1) * 128, :]
                    )
                return at_dma[k]

            def moving(g, k, b):
                if g == "h" and 8 <= k < 16:
                    return rh_t[k - 8][:, b * 512:(b + 1) * 512]
                return at_t[k][:, b * 512:(b + 1) * 512]

            def do_group(g, jq):
                ps = {}
                for jl in range(2):
                    for b in range(NB):
                        ps[(jl, b)] = psp.tile([128, 512], f32, tag="ps",
                                               name=f"ps_{g}_{jq}_{jl}_{b}")
                for ks in range(KT // KSLAB):
                    k0 = KSLAB * ks
                    slab = wp.tile([128, KSLAB, 256], f16, tag="w", name=f"w_{g}_{jq}_{ks}")
                    src = w_d[g].ap()[k0 * 128:(k0 + KSLAB) * 128,
                                      jq * 256:(jq + 1) * 256]
                    nc.sync.dma_start(slab[:], src.rearrange("(a p) j -> p a j", p=128))
                    for dk in range(KSLAB):
                        k = k0 + dk
                        if not (g == "h" and 8 <= k < 16):
                            ensure_at(k)
                        for jl in range(2):
                            lhsT = slab[:, dk, jl * 128:(jl + 1) * 128]
                            for b in range(NB):
                                nc.tensor.matmul(
                                    ps[(jl, b)][:],
                                    lhsT,
                                    moving(g, k, b),
                                    start=(k == 0),
                                    stop=(k == KT - 1),
                                )
                for jl in range(2):
                    jt = 2 * jq + jl
                    for b in range(NB):
                        pst = ps[(jl, b)]
                        bias_ap = bias_t[g][:, jt:jt + 1]
                        bsl = slice(b * 512, (b + 1) * 512)
                        if g == "r":
                            tmp = rp.tile([128, 512], f32, tag="rt", name=f"rt_{jt}_{b}")
                            nc.scalar.activation(tmp[:], pst[:], SIG, bias=bias_ap)
                            nc.vector.tensor_mul(
                                rh_t[jt][:, bsl], tmp[:], at_t[8 + jt][:, bsl]
                            )
                        elif g == "z":
                            nc.scalar.activation(z_t[jt][b][:], pst[:], SIG, bias=bias_ap)
                        else:
                            hp = hpp.tile([128, 512], f32, tag="hp", name=f"hp_{jt}_{b}")
                            nc.scalar.activation(hp[:], pst[:], TANH, bias=bias_ap)
                            hT = at_t[8 + jt][:, bsl]
                            ot = op.tile([128, 512], f32, tag="ot", name=f"ot_{jt}_{b}")
                            nc.vector.tensor_sub(ot[:], hp[:], hT)
                            nc.vector.tensor_mul(hp[:], z_t[jt][b][:], ot[:])
                            nc.vector.tensor_add(ot[:], hp[:], hT)
                            nc.sync.dma_start(
                                out_d.ap()[jt * 128:(jt + 1) * 128, bsl], ot[:]
                            )

            for g in ("r", "z", "h"):
                for jq in range(4):
                    do_group(g, jq)

    _dedup_ldweights(nc, mybir)
    nc.finalize()
    return nc


def _get_nc():
    if "nc" not in _CACHE:
        _CACHE["nc"] = _build_nc_fp8dr() if MM_MODE == "fp8dr" else _build_nc_f16()
    return _CACHE["nc"]


def _fp8_split(arr32):
    import ml_dtypes

    hi = arr32.astype(ml_dtypes.float8_e4m3fn)
    lo = (arr32 - hi.astype(np.float32)).astype(ml_dtypes.float8_e4m3fn)
    return hi, lo


def _plane_interleave(arr, ncols):
    # [4096, N] -> [KP, 128, 2, N] with plane = k-tile parity within the pair
    return np.ascontiguousarray(
        arr.reshape(KP, 2, 128, ncols).transpose(0, 2, 1, 3)
    )


def _pack_weight_slabs(arr):
    # [4096, 1024] -> [jq, ks, p, dkp, pl, jcol] slab-contiguous layout
    a = _plane_interleave(arr, H)                       # [KP, 128, 2, H]
    a = a.reshape(KP // KSLAB, KSLAB, 128, 2, 4, 256)   # [ks, dkp, p, pl, jq, jcol]
    return np.ascontiguousarray(a.transpose(4, 0, 2, 1, 3, 5))


def _host_prep(inputs):
    x = np.asarray(inputs["x"], dtype=np.float32)
    hprev = np.asarray(inputs["hprev"], dtype=np.float32)
    c = np.asarray(inputs["c"], dtype=np.float32)
    A = np.concatenate([x, hprev, c], axis=1)          # [B, 4096]
    AT = np.ascontiguousarray(A.T)                     # [4096, B] f32
    wT = {}
    for g, (W, U, C) in {
        "r": (inputs["Wr"], inputs["Ur"], inputs["Cr"]),
        "z": (inputs["Wz"], inputs["Uz"], inputs["Cz"]),
        "h": (inputs["Wh"], inputs["Uh"], inputs["Ch"]),
    }.items():
        M = np.concatenate(
            [np.asarray(W, np.float32), np.asarray(U, np.float32), np.asarray(C, np.float32)],
            axis=1,
        )
        wT[g] = np.ascontiguousarray(M.T)              # [4096, 1024] f32
    bias = {
        g: np.ascontiguousarray(
            np.asarray(inputs["b" + g], dtype=np.float32).reshape(8, 128).T
        )
        for g in "rzh"
    }
    return AT, wT, bias, hprev


def _in_maps(inputs):
    AT, wT, bias, hprev = _host_prep(inputs)
    maps = []
    if MM_MODE == "fp8dr":
        w_split = {}
        for g in "rzh":
            hi, lo = _fp8_split(wT[g])
            w_split[g] = (_pack_weight_slabs(hi), _pack_weight_slabs(lo))
        hT_full = np.ascontiguousarray(hprev.T.astype(np.float16))  # [1024, B]
        at_hi_f, at_lo_f = _fp8_split(AT)
        for s in range(NCORES):
            sl = slice(s * BS, (s + 1) * BS)
            m = {
                "at_hi": _plane_interleave(np.ascontiguousarray(at_hi_f[:, sl]), BS),
                "at_lo": _plane_interleave(np.ascontiguousarray(at_lo_f[:, sl]), BS),
                "ht": np.ascontiguousarray(hT_full[:, sl]).reshape(8, 128, BS),
                "br": bias["r"],
                "bz": bias["z"],
                "bh": bias["h"],
            }
            for g in "rzh":
                m[f"w{g}_hi"], m[f"w{g}_lo"] = w_split[g]
            maps.append(m)
    else:
        w16 = {g: wT[g].astype(np.float16) for g in "rzh"}
        AT16 = AT.astype(np.float16)
        for s in range(NCORES):
            maps.append(
                {
                    "at": np.ascontiguousarray(AT16[:, s * BS:(s + 1) * BS]),
                    "wr": w16["r"],
                    "wz": w16["z"],
                    "wh": w16["h"],
                    "br": bias["r"],
                    "bz": bias["z"],
                    "bh": bias["h"],
                }
            )
    return maps


def run_device(inputs, trace=False, **kwargs):
    """Run the SPMD kernel; returns (full_output, BassKernelResults)."""
    from concourse.bass_utils import run_bass_kernel_spmd

    nc = _get_nc()
    res = run_bass_kernel_spmd(
        nc, _in_maps(inputs), core_ids=list(range(NCORES)), trace=trace, **kwargs
    )
    out = np.empty((B, H), dtype=np.float32)
    for s in range(NCORES):
        out[s * BS:(s + 1) * BS, :] = res.results[s]["out_t"].T
    return out, res


def kernel(**inputs):
    out, _ = run_device(inputs, trace=False)
    return out
